# revision 1
# baseline (speedup 1.0000x reference)
"""Expert-parallel MoE kernel for Trainium2 (8 NeuronCores).

Strategy (per spec sharding hint): one expert per core. The router
(softmax top-2 over E=8) is computed on host as part of the token
dispatch: tokens routed to expert e are gathered into a contiguous,
capacity-padded buffer and shipped (transposed, bf16) to core e
together with that expert's weights. Each core runs the SwiGLU-style
FFN (relu gate) for its tokens:

    yT = w2 @ (relu(w1 @ xT) * (w3 @ xT))        [D, C]

The host applies the routing weights and scatter-adds the per-expert
outputs back into the full [B, S, D] output.

Shapes: B=4, S=4096, D=512, F=2048, E=8, top_k=2, T=B*S=16384.
Capacity C is derived from the actual routing (rounded up to a
multiple of 512); all cores share one SPMD program.
"""

import numpy as np
import ml_dtypes

BF16 = ml_dtypes.bfloat16

D = 512
F = 2048
E = 8
KD = D // 128   # 4  D-subtiles (contraction for GEMM 1, output for GEMM 2)
KF = F // 128   # 16 F-subtiles (output for GEMM 1, contraction for GEMM 2)
NFREE = 512     # matmul free-dim / C-chunk width

_PROGRAM_CACHE: dict = {}


def build_program(C: int, repeats: int = 1):
    """Build + finalize the per-core Bass program for capacity C."""
    import concourse.bacc as bacc
    import concourse.mybir as mybir
    import concourse.tile as tile

    bf16 = mybir.dt.bfloat16
    f32 = mybir.dt.float32
    n_chunks = C // NFREE
    assert C % NFREE == 0

    nc = bacc.Bacc()
    xt_d = nc.declare_dram_parameter("xt", [KD, 128, C], bf16, isOutput=False)
    w1_d = nc.declare_dram_parameter("w1t", [KD, 128, F], bf16, isOutput=False)
    w3_d = nc.declare_dram_parameter("w3t", [KD, 128, F], bf16, isOutput=False)
    w2_d = nc.declare_dram_parameter("w2t", [KF, 128, D], bf16, isOutput=False)
    yt_d = nc.declare_dram_parameter("yt", [KD, 128, C], f32, isOutput=True)

    with tile.TileContext(nc) as tc:
        with (
            tc.tile_pool(name="weights", bufs=1) as wpool,
            tc.tile_pool(name="xpool", bufs=1) as xpool,
            tc.tile_pool(name="gpool", bufs=2) as gpool,
            tc.tile_pool(name="hpool", bufs=3) as hpool,
            tc.tile_pool(name="ypool", bufs=3) as ypool,
            tc.tile_pool(name="psum", bufs=2, space="PSUM") as psum,
        ):
            w1_sb = wpool.tile([128, KD, F], bf16)
            w3_sb = wpool.tile([128, KD, F], bf16)
            w2_sb = wpool.tile([128, KF, D], bf16)
            xt_sb = xpool.tile([128, KD, C], bf16)
            for kd in range(KD):
                nc.sync.dma_start(w1_sb[:, kd, :], w1_d[kd])
                nc.sync.dma_start(w3_sb[:, kd, :], w3_d[kd])
                nc.sync.dma_start(xt_sb[:, kd, :], xt_d[kd])
            for kf in range(KF):
                nc.sync.dma_start(w2_sb[:, kf, :], w2_d[kf])

            for _rep in range(repeats):
                for c in range(n_chunks):
                    cs = slice(c * NFREE, (c + 1) * NFREE)
                    g_sb = gpool.tile([128, KF, NFREE], bf16, name="g")
                    for kf in range(KF):
                        fs = slice(kf * 128, (kf + 1) * 128)
                        ph1 = psum.tile([128, NFREE], f32, name="ph1")
                        ph3 = psum.tile([128, NFREE], f32, name="ph3")
                        for kd in range(KD):
                            nc.tensor.matmul(
                                ph1, w1_sb[:, kd, fs], xt_sb[:, kd, cs],
                                start=(kd == 0), stop=(kd == KD - 1),
                            )
                        for kd in range(KD):
                            nc.tensor.matmul(
                                ph3, w3_sb[:, kd, fs], xt_sb[:, kd, cs],
                                start=(kd == 0), stop=(kd == KD - 1),
                            )
                        h1_sb = hpool.tile([128, NFREE], f32, name="h1")
                        nc.scalar.activation(
                            h1_sb[:], ph1[:], mybir.ActivationFunctionType.Relu,
                        )
                        nc.vector.tensor_tensor(
                            g_sb[:, kf, :], h1_sb[:], ph3[:],
                            mybir.AluOpType.mult,
                        )
                    for dt_i in range(KD):
                        ds_ = slice(dt_i * 128, (dt_i + 1) * 128)
                        py = psum.tile([128, NFREE], f32, name="py")
                        for kf in range(KF):
                            nc.tensor.matmul(
                                py, w2_sb[:, kf, ds_], g_sb[:, kf, :],
                                start=(kf == 0), stop=(kf == KF - 1),
                            )
                        y_sb = ypool.tile([128, NFREE], f32, name="y")
                        nc.any.tensor_copy(y_sb[:], py[:])
                        nc.sync.dma_start(yt_d[dt_i, :, cs], y_sb[:])

    nc.finalize()
    return nc


def route(x2d: np.ndarray, gate_w: np.ndarray, top_k: int):
    """Replicate the reference router in numpy (fp32).

    Returns sel [T, k] int64, rw [T, k] fp32 (renormalized)."""
    logits = x2d @ gate_w.T                      # [T, E] fp32
    m = logits.max(axis=-1, keepdims=True)
    p = np.exp(logits - m, dtype=np.float32)
    p /= p.sum(axis=-1, keepdims=True)
    # top-k, ties -> lowest index (matches jax.lax.top_k)
    sel = np.argsort(-p, axis=-1, kind="stable")[:, :top_k]
    rw = np.take_along_axis(p, sel, axis=-1)
    rw = rw / rw.sum(axis=-1, keepdims=True)
    return sel, rw.astype(np.float32)


def kernel(x, gate_w, w1, w2, w3, top_k):
    from concourse.bass_utils import run_bass_kernel_spmd

    x = np.asarray(x, dtype=np.float32)
    gate_w = np.asarray(gate_w, dtype=np.float32)
    w1 = np.asarray(w1, dtype=np.float32)
    w2 = np.asarray(w2, dtype=np.float32)
    w3 = np.asarray(w3, dtype=np.float32)
    k = int(top_k)

    B, S, Dx = x.shape
    assert Dx == D and w1.shape[0] == E
    T = B * S
    x2d = x.reshape(T, D)

    sel, rw = route(x2d, gate_w, k)

    # per-expert token lists
    idx_list, cw_list = [], []
    for e in range(E):
        tok, kk = np.nonzero(sel == e)
        idx_list.append(tok)
        cw_list.append(rw[tok, kk])
    counts = np.array([len(i) for i in idx_list])
    C = max(int(np.max(counts)), NFREE)
    C = ((C + NFREE - 1) // NFREE) * NFREE

    key = C
    if key not in _PROGRAM_CACHE:
        _PROGRAM_CACHE[key] = build_program(C)
    nc = _PROGRAM_CACHE[key]

    in_maps = []
    for e in range(E):
        idx = idx_list[e]
        xg = np.zeros((C, D), dtype=np.float32)
        xg[: len(idx)] = x2d[idx]
        xt = np.ascontiguousarray(xg.T).astype(BF16).reshape(KD, 128, C)
        w1t = np.ascontiguousarray(w1[e].T).astype(BF16).reshape(KD, 128, F)
        w3t = np.ascontiguousarray(w3[e].T).astype(BF16).reshape(KD, 128, F)
        w2t = np.ascontiguousarray(w2[e].T).astype(BF16).reshape(KF, 128, D)
        in_maps.append({"xt": xt, "w1t": w1t, "w3t": w3t, "w2t": w2t})

    res = run_bass_kernel_spmd(nc, in_maps, list(range(E)))

    out = np.zeros((T, D), dtype=np.float32)
    for e in range(E):
        idx = idx_list[e]
        yt = res.results[e]["yt"]               # [KD, 128, C] f32
        y = yt.reshape(D, C)[:, : len(idx)].T   # [n_e, D]
        out[idx] += cw_list[e][:, None] * y
    return out.reshape(B, S, D)
